# revision 22
# baseline (speedup 1.0000x reference)
"""Trainium2 Bass kernel for nn_CPAMDec_Mix (dual cross-attention decoder block).

Math per batch sample b (C=512, C4=128, K=64, N=W*H=4096):
    pv1 = wv @ y1^T + bv          [C, K]
    pv2 = wv @ y2^T + bv          [C, K]
    q^T = wq @ x2 + bq            [C4, N]
    kk  = y2 @ wk^T + bk          [K, C4]
    energy = q @ kk^T             [N, K]
    att = softmax(|energy|, -1)   [N, K]
    out1 = scale  * pv1 @ att^T + x1
    out2 = scale1 * pv2 @ att^T + x2

Sharding: pure data parallel - sample b on core b (B == n_cores == 8).

The kernel is HBM-bound, so all large tensors move as fp16 (x1, x2, the
weights, and both outputs), halving DRAM traffic vs f32. fp16 keeps a
10-bit mantissa - the same precision the f32r (TF32-like) matmuls of an
f32 variant would have - so end-to-end error stays ~4e-4 l2. Matmul
operands are fp16 (1 cycle/col on PE), accumulation stays fp32 in PSUM,
softmax runs in fp32. scale/scale1 fold into the pv tiles so the output
epilogue is one add per tile.

Engine balance (the 8-core power governor throttles the PE to ~1.2 GHz
mid-kernel, so PE cycles pace the steady state; every other engine is
kept below that pace):
  - PE: q-projection, energy, att transposes, output GEMMs.
  - DVE: softmax (abs-mask, reduce, reciprocal, one broadcast multiply
    per n-tile) + the out1 residual adds and one out2 add per quarter,
    each as a [128, 1024] PSUM-pair op (PSUM + fp16 residual -> fp16).
  - ACT: qT/exp/attT copies + 3 of 4 out2 PSUM->fp16 copies per quarter.
  - GPSIMD: x1 prefetch (SWDGE) + the matching out2 residual adds
    (SBUF-only, since GpSimd has no PSUM port).

Software pipeline over 4 column-quarters of N; attention is emitted one
quarter ahead of the output phase so neither PE nor ACT stalls at
quarter boundaries. PSUM budget: psq 2 + pse 2x0.5 + pst 2x0.5 + two
[128, 1024] output-pair buffers = 8 banks.

DMA: the 16 SDMA engines round-robin ALL in-flight DMAs at packet
granularity, so transfers are ordered by need-time: wqT and x2[0] load
first and nearly alone (x1[0] is gated behind an x2[0]-dependent no-op
on the gpsimd queue), and later x2/x1 quarters prefetch inside the
loop. x1/x2/outs are host-packed as [128, (q, cc, n)] so every quarter
moves as one contiguous 8KB-per-partition DMA (128 descriptors).
Stores drain per half-quarter on the sync (out1) and scalar (out2)
HWDGE rings, with the final quarter split finer to shrink the tail.
"""

import numpy as np

import concourse.bass as bass
import concourse.mybir as mybir
import concourse.tile as tile
from concourse import bacc
from concourse.bass_utils import run_bass_kernel_spmd
from concourse.masks import make_identity

F32 = mybir.dt.float32
F16 = mybir.dt.float16
AX = mybir.AxisListType
OP = mybir.AluOpType
AF = mybir.ActivationFunctionType

B, C, W, H, K = 8, 512, 64, 64, 64
C4 = C // 4
N = W * H            # 4096
NT = 512             # n-tile (columns per matmul / psum bank)
NQ = 1024            # quarter width (x1/x2 DMA chunk)
CC = C // 128        # 4 chunks of 128 over the channel dim

_CACHE = {}


def _build_nc():
    nc = bacc.Bacc("TRN2", target_bir_lowering=False, debug=False)

    # x1/x2/outs host-packed as [128, (q, cc, n)] so each quarter is one
    # contiguous [128, 8KB] block (128 descriptors per DMA, no rearrange)
    x1_d = nc.dram_tensor("x1h", [128, CC * N], F16, kind="ExternalInput")
    x2_d = nc.dram_tensor("x2h", [128, CC * N], F16, kind="ExternalInput")
    # host pre-packed to the SBUF chunk layout [128, CC*inner]
    y1T_d = nc.dram_tensor("y1T", [128, CC * K], F16, kind="ExternalInput")
    y2T_d = nc.dram_tensor("y2T", [128, CC * K], F16, kind="ExternalInput")
    wqT_d = nc.dram_tensor("wqT", [128, CC * C4], F16, kind="ExternalInput")
    wkT_d = nc.dram_tensor("wkT", [128, CC * C4], F16, kind="ExternalInput")
    wvT_d = nc.dram_tensor("wvT", [128, CC * C], F16, kind="ExternalInput")
    # packed per-partition vectors: [bq | bk | scale | scale1]
    vecs_d = nc.dram_tensor("vecs", [C4, 4], F32, kind="ExternalInput")
    # packed rows: [bv (512) | ones (64)]
    rows_d = nc.dram_tensor("rows", [1, C + K], F16, kind="ExternalInput")
    out1_d = nc.dram_tensor("out1", [128, CC * N], F16, kind="ExternalOutput")
    out2_d = nc.dram_tensor("out2", [128, CC * N], F16, kind="ExternalOutput")

    with tile.TileContext(nc) as tc:
        with (
            tc.tile_pool(name="const", bufs=1) as const,
            tc.tile_pool(name="qpool", bufs=2) as qpool,
            tc.tile_pool(name="spool", bufs=3) as spool,
            tc.tile_pool(name="apool", bufs=4) as apool,
            tc.tile_pool(name="x1pool", bufs=2) as x1pool,
            tc.tile_pool(name="o1pool", bufs=2) as o1pool,
            tc.tile_pool(name="o2pool", bufs=2) as o2pool,
            tc.tile_pool(name="t2pool", bufs=3) as t2pool,
            tc.tile_pool(name="psq", bufs=2, space="PSUM") as psq,
            tc.tile_pool(name="pse", bufs=1, space="PSUM") as pse,
            tc.tile_pool(name="pst", bufs=1, space="PSUM") as pst,
            tc.tile_pool(name="pso", bufs=2, space="PSUM") as pso,
        ):
            # ---- small weights first: everything in flight round-robins
            # ---- on the SDMA engines, so bulk loads must not lead ----
            wqT_sb = const.tile([128, CC * C4], F16)
            nc.sync.dma_start(out=wqT_sb[:], in_=wqT_d[:])
            x2_sb = []
            x2q0 = const.tile([128, CC * NQ], F16, tag="x2_0", name="x2q0")
            nc.sync.dma_start(out=x2q0[:], in_=x2_d[:, 0 : CC * NQ])
            x2_sb.append(x2q0)
            wkT_sb = const.tile([128, CC * C4], F16)
            nc.sync.dma_start(out=wkT_sb[:], in_=wkT_d[:])
            y2T_sb = const.tile([128, CC * K], F16)
            nc.sync.dma_start(out=y2T_sb[:], in_=y2T_d[:])
            vecs_sb = const.tile([C4, 4], F32)
            nc.sync.dma_start(out=vecs_sb[:], in_=vecs_d[:])
            bq_sb = vecs_sb[:, 0:1]
            bk_sb = vecs_sb[:, 1:2]
            sc1_sb = vecs_sb[0:K, 2:3]
            sc2_sb = vecs_sb[0:K, 3:4]
            rows_sb = const.tile([1, C + K], F16)
            nc.sync.dma_start(out=rows_sb[:], in_=rows_d[:])
            bv_sb = rows_sb[:, 0:C]
            ones_sb = rows_sb[:, C : C + K]
            ident = const.tile([128, 128], F16)
            make_identity(nc, ident[:])

            # x1 quarter 0 (gpsimd SWDGE ring): gated behind a 1-element
            # copy that depends on x2[0], so its packets don't round-robin-
            # steal HBM bandwidth from the startup-critical x2[0] load
            # (x1 is only needed once quarter 0's epilogue begins)
            gate = const.tile([1, 1], F16, name="gate")
            nc.gpsimd.tensor_copy(gate[:], x2q0[0:1, 0:1])
            x1_sb = {}
            x1_sb[0] = x1pool.tile([128, CC * NQ], F16, tag="x1t", name="x1t")
            nc.gpsimd.dma_start(out=x1_sb[0][:], in_=x1_d[:, 0 : CC * NQ])

            # ---- kk^T, then a PE warm-up burst while x2[0] is in flight ----
            pkk = pse.tile([C4, K], F32, tag="pse")
            for cc in range(CC):
                nc.tensor.matmul(
                    pkk[:],
                    lhsT=wkT_sb[:, cc * C4 : (cc + 1) * C4],
                    rhs=y2T_sb[:, cc * K : (cc + 1) * K],
                    start=(cc == 0),
                    stop=(cc == CC - 1),
                )
            kkT_sb = const.tile([C4, K], F16)
            nc.scalar.activation(kkT_sb[:], pkk[:], AF.Identity, bias=bk_sb)

            # value-path weights + later x2 quarters issue from the ACT ring
            # AFTER compute ops, so the sequencer naturally holds them back
            # and their packets don't round-robin-steal DMA bandwidth from
            # the startup-critical x2[0]/wqT loads
            wvT_sb = const.tile([128, CC * C], F16)
            nc.scalar.dma_start(out=wvT_sb[:], in_=wvT_d[:])
            y1T_sb = const.tile([128, CC * K], F16)
            nc.scalar.dma_start(out=y1T_sb[:], in_=y1T_d[:])

            pv_sb = []

            def emit_pv():
                # pv1^T, pv2^T: [K, C] = y^T.T @ wvT (+ ones^T bv), scaled.
                # Emitted after quarter 0's attention so the PE doesn't block
                # on the wvT load before starting the q-projection.
                for yT_sb, sc in ((y1T_sb, sc1_sb), (y2T_sb, sc2_sb)):
                    ppv = pso.tile([K, C], F32, tag="pso", name="ppv")
                    for cc in range(CC):
                        nc.tensor.matmul(
                            ppv[:],
                            lhsT=yT_sb[:, cc * K : (cc + 1) * K],
                            rhs=wvT_sb[:, cc * C : (cc + 1) * C],
                            start=(cc == 0),
                            stop=False,
                        )
                    nc.tensor.matmul(
                        ppv[:], lhsT=ones_sb, rhs=bv_sb, start=False, stop=True
                    )
                    pv = const.tile([K, C], F16, tag=f"pv_{len(pv_sb)}", name="pv")
                    nc.scalar.activation(pv[:], ppv[:], AF.Copy, scale=sc)
                    pv_sb.append(pv)

            # ---- pipeline over quarters ----
            for q in range(N // NQ):
                x2q = x2_sb[q]
                # prefetch next x1 quarter on the gpsimd queue
                if q + 1 < N // NQ:
                    x1_sb[q + 1] = x1pool.tile(
                        [128, CC * NQ], F16, tag="x1t", name="x1t"
                    )
                    nc.gpsimd.dma_start(
                        out=x1_sb[q + 1][:],
                        in_=x1_d[:, (q + 1) * CC * NQ : (q + 2) * CC * NQ],
                    )

                # -- attention: q-projection for both halves, cc-outer --
                psum_q = [
                    psq.tile([C4, NT], F32, tag="psq", name="psum_q")
                    for _ in range(2)
                ]
                for cc in range(CC):
                    for h in range(2):
                        nc.tensor.matmul(
                            psum_q[h][:],
                            lhsT=wqT_sb[:, cc * C4 : (cc + 1) * C4],
                            rhs=x2q[:, cc * NQ + h * NT : cc * NQ + h * NT + NT],
                            start=(cc == 0),
                            stop=(cc == CC - 1),
                        )
                if q + 1 < N // NQ:
                    t = const.tile(
                        [128, CC * NQ], F16, tag=f"x2_{q + 1}", name="x2t"
                    )
                    nc.scalar.dma_start(
                        out=t[:], in_=x2_d[:, (q + 1) * CC * NQ : (q + 2) * CC * NQ]
                    )
                    x2_sb.append(t)
                aTs = []
                for h in range(2):
                    qT = qpool.tile([C4, NT], F16, tag="qT")
                    nc.scalar.activation(qT[:], psum_q[h][:], AF.Identity, bias=bq_sb)
                    # energy [n, k] in 128-row chunks: qT_slice.T @ kkT
                    psum_e = pse.tile([128, 4 * K], F32, tag="pse")
                    for s in range(4):
                        nc.tensor.matmul(
                            psum_e[:, s * K : (s + 1) * K],
                            lhsT=qT[:, s * 128 : (s + 1) * 128],
                            rhs=kkT_sb[:],
                            start=True,
                            stop=True,
                        )
                    # softmax(|e|) along k (free dim), no max-subtraction:
                    # |e| <= ~20 here so exp is safely in fp32 range.
                    eexp = spool.tile([128, 4 * K], F32, tag="eexp")
                    nc.vector.tensor_scalar(
                        eexp[:].bitcast(mybir.dt.uint32),
                        psum_e[:].bitcast(mybir.dt.uint32),
                        0x7FFFFFFF,
                        None,
                        op0=OP.bitwise_and,
                    )
                    nc.scalar.activation(eexp[:], eexp[:], AF.Exp)
                    rsum = spool.tile([128, 4], F32, tag="rsum")
                    nc.vector.tensor_reduce(
                        rsum[:],
                        eexp[:].rearrange("p (g d) -> p g d", g=4),
                        axis=AX.X,
                        op=OP.add,
                    )
                    rrec = spool.tile([128, 4], F32, tag="rrec")
                    nc.vector.reciprocal(rrec[:], rsum[:])
                    att = spool.tile([128, 4 * K], F16, tag="att")
                    nc.vector.tensor_tensor(
                        att[:].rearrange("p (g d) -> p g d", g=4),
                        eexp[:].rearrange("p (g d) -> p g d", g=4),
                        rrec[:]
                        .rearrange("p (g o) -> p g o", g=4)
                        .broadcast_to([128, 4, K]),
                        op=OP.mult,
                    )
                    # transpose att [n,k] -> attT [k,n] (fp16 PE transpose)
                    psum_t = pst.tile([K, NT], F16, tag="pst")
                    for s in range(4):
                        nc.tensor.transpose(
                            psum_t[:, s * 128 : (s + 1) * 128],
                            att[:, s * K : (s + 1) * K],
                            ident[:],
                        )
                    aT = apool.tile([K, NT], F16, tag="attT")
                    nc.scalar.copy(aT[:], psum_t[:])
                    aTs.append(aT)

                if q == 0:
                    emit_pv()
                pv1T_sb, pv2T_sb = pv_sb

                # -- output GEMMs + epilogue --
                # PSUM pairs [128, 2*NT] so each epilogue op covers a full NQ.
                # out1: DVE add (psum + x1 -> fp16).  out2: one pair per
                # quarter on DVE, the rest ACT copy + GPSIMD add (keeps DVE,
                # ACT and GpSimd all below the PE wall).
                o1 = o1pool.tile([128, CC * NQ], F16, tag="o1")
                o2 = o2pool.tile([128, CC * NQ], F16, tag="o2")
                for cc in range(CC):
                    o = cc * NQ
                    pos = []
                    for pvT in (pv1T_sb, pv2T_sb):
                        po = pso.tile([128, 2 * NT], F32, tag="pso", name="po")
                        for h in range(2):
                            nc.tensor.matmul(
                                po[:, h * NT : (h + 1) * NT],
                                lhsT=pvT[:, cc * 128 : (cc + 1) * 128],
                                rhs=aTs[h][:],
                                start=True,
                                stop=True,
                            )
                        pos.append(po)
                    nc.vector.tensor_add(
                        o1[:, o : o + NQ], pos[0][:], x1_sb[q][:, o : o + NQ]
                    )
                    if cc == 3:
                        nc.vector.tensor_add(
                            o2[:, o : o + NQ], pos[1][:], x2_sb[q][:, o : o + NQ]
                        )
                    else:
                        t2 = t2pool.tile([128, 2 * NT], F16, tag="t2")
                        nc.scalar.copy(t2[:], pos[1][:])
                        nc.gpsimd.tensor_add(
                            o2[:, o : o + NQ], t2[:], x2_sb[q][:, o : o + NQ]
                        )
                    if cc == 1 or cc == 3:
                        g = cc // 2  # store half-quarters as they complete
                        base = q * CC * NQ + 2 * g * NQ
                        nc.sync.dma_start(
                            out=out1_d[:, base : base + 2 * NQ],
                            in_=o1[:, 2 * g * NQ : (2 * g + 2) * NQ],
                        )
                        nc.scalar.dma_start(
                            out=out2_d[:, base : base + 2 * NQ],
                            in_=o2[:, 2 * g * NQ : (2 * g + 2) * NQ],
                        )
    nc.compile()
    return nc


def _get_nc():
    if "nc" not in _CACHE:
        _CACHE["nc"] = _build_nc()
    return _CACHE["nc"]


def _chunked(a):
    """[C, inner] -> [128, CC*inner] host pack (row chunk cc at cols cc*inner)."""
    c, inner = a.shape
    return np.ascontiguousarray(
        a.reshape(CC, 128, inner).transpose(1, 0, 2).reshape(128, CC * inner)
    )


def kernel(x1, y1, x2, y2, wq, bq, wk, bk, wv, bv, scale, scale1, **run_kwargs):
    x1 = np.asarray(x1, np.float32)
    x2 = np.asarray(x2, np.float32)
    vecs = np.stack(
        [
            np.asarray(bq, np.float32).reshape(C4),
            np.asarray(bk, np.float32).reshape(C4),
            np.full(C4, np.asarray(scale).reshape(-1)[0], np.float32),
            np.full(C4, np.asarray(scale1).reshape(-1)[0], np.float32),
        ],
        axis=1,
    )
    rows = np.concatenate(
        [np.asarray(bv, np.float16).reshape(C), np.ones(K, np.float16)]
    ).reshape(1, C + K)
    shared = {
        "wqT": _chunked(np.asarray(wq, np.float32).T.astype(np.float16)),
        "wkT": _chunked(np.asarray(wk, np.float32).T.astype(np.float16)),
        "wvT": _chunked(np.asarray(wv, np.float32).T.astype(np.float16)),
        "vecs": np.ascontiguousarray(vecs),
        "rows": np.ascontiguousarray(rows).astype(np.float16),
    }
    def _qpack(a):
        # [C, N] -> [128, (q, cc, n)]: each quarter contiguous per partition
        return np.ascontiguousarray(
            a.reshape(CC, 128, N // NQ, NQ).transpose(1, 2, 0, 3).reshape(128, CC * N)
        )

    in_maps = []
    for b in range(B):
        in_maps.append(
            {
                "x1h": _qpack(x1[b].reshape(C, N).astype(np.float16)),
                "x2h": _qpack(x2[b].reshape(C, N).astype(np.float16)),
                "y1T": _chunked(np.asarray(y1[b], np.float32).T.astype(np.float16)),
                "y2T": _chunked(np.asarray(y2[b], np.float32).T.astype(np.float16)),
                **shared,
            }
        )
    nc = _get_nc()
    res = run_bass_kernel_spmd(nc, in_maps, list(range(B)), **run_kwargs)
    _CACHE["last_results"] = res

    def _qunpack(a):
        # [128, (q, cc, n)] -> [C, W, H]
        return (
            a.reshape(128, N // NQ, CC, NQ)
            .transpose(2, 0, 1, 3)
            .reshape(C, N)
            .astype(np.float32)
            .reshape(C, W, H)
        )

    out1 = np.stack([_qunpack(res.results[b]["out1"]) for b in range(B)])
    out2 = np.stack([_qunpack(res.results[b]["out2"]) for b in range(B)])
    return (out1, out2)


# revision 23
# speedup vs baseline: 1.0252x; 1.0252x over previous
"""Trainium2 Bass kernel for nn_CPAMDec_Mix (dual cross-attention decoder block).

Math per batch sample b (C=512, C4=128, K=64, N=W*H=4096):
    pv1 = wv @ y1^T + bv          [C, K]
    pv2 = wv @ y2^T + bv          [C, K]
    q^T = wq @ x2 + bq            [C4, N]
    kk  = y2 @ wk^T + bk          [K, C4]
    energy = q @ kk^T             [N, K]
    att = softmax(|energy|, -1)   [N, K]
    out1 = scale  * pv1 @ att^T + x1
    out2 = scale1 * pv2 @ att^T + x2

Sharding: pure data parallel - sample b on core b (B == n_cores == 8).

The kernel is HBM-bound, so all large tensors move as fp16 (x1, x2, the
weights, and both outputs), halving DRAM traffic vs f32. fp16 keeps a
10-bit mantissa - the same precision the f32r (TF32-like) matmuls of an
f32 variant would have - so end-to-end error stays ~4e-4 l2. Matmul
operands are fp16 (1 cycle/col on PE), accumulation stays fp32 in PSUM,
softmax runs in fp32. scale/scale1 fold into the pv tiles so the output
epilogue is one add per tile.

Engine balance (the 8-core power governor throttles the PE to ~1.2 GHz
mid-kernel, so PE cycles pace the steady state; every other engine is
kept below that pace):
  - PE: q-projection, energy, att transposes, output GEMMs.
  - DVE: softmax (abs-mask, reduce, reciprocal, one broadcast multiply
    per n-tile) + the out1 residual adds and one out2 add per quarter,
    each as a [128, 1024] PSUM-pair op (PSUM + fp16 residual -> fp16).
  - ACT: qT/exp/attT copies + 3 of 4 out2 PSUM->fp16 copies per quarter.
  - GPSIMD: x1 prefetch (SWDGE) + the matching out2 residual adds
    (SBUF-only, since GpSimd has no PSUM port).

Software pipeline over 4 column-quarters of N; attention is emitted one
quarter ahead of the output phase so neither PE nor ACT stalls at
quarter boundaries. PSUM budget: psq 2 + pse 2x0.5 + pst 2x0.5 + two
[128, 1024] output-pair buffers = 8 banks.

DMA: the 16 SDMA engines round-robin ALL in-flight DMAs at packet
granularity, so transfers are ordered by need-time: wqT and x2[0] load
first and nearly alone (x1[0] is gated behind an x2[0]-dependent no-op
on the gpsimd queue), and later x2/x1 quarters prefetch inside the
loop. x1/x2/outs are host-packed as [128, (q, cc, n)] so every quarter
moves as one contiguous 8KB-per-partition DMA (128 descriptors).
Stores drain per half-quarter on the sync (out1) and scalar (out2)
HWDGE rings, with the final quarter split finer to shrink the tail.
"""

import numpy as np

import concourse.bass as bass
import concourse.mybir as mybir
import concourse.tile as tile
from concourse import bacc
from concourse.bass_utils import run_bass_kernel_spmd
from concourse.masks import make_identity

F32 = mybir.dt.float32
F16 = mybir.dt.float16
AX = mybir.AxisListType
OP = mybir.AluOpType
AF = mybir.ActivationFunctionType

B, C, W, H, K = 8, 512, 64, 64, 64
C4 = C // 4
N = W * H            # 4096
NT = 512             # n-tile (columns per matmul / psum bank)
NQ = 1024            # quarter width (x1/x2 DMA chunk)
CC = C // 128        # 4 chunks of 128 over the channel dim

_CACHE = {}


def _build_nc():
    nc = bacc.Bacc("TRN2", target_bir_lowering=False, debug=False)

    # x1/x2/outs host-packed as [128, (q, cc, n)] so each quarter is one
    # contiguous [128, 8KB] block (128 descriptors per DMA, no rearrange)
    x1_d = nc.dram_tensor("x1h", [128, CC * N], F16, kind="ExternalInput")
    x2_d = nc.dram_tensor("x2h", [128, CC * N], F16, kind="ExternalInput")
    # host pre-packed to the SBUF chunk layout [128, CC*inner]
    y1T_d = nc.dram_tensor("y1T", [128, CC * K], F16, kind="ExternalInput")
    wqT_d = nc.dram_tensor("wqT", [128, CC * C4], F16, kind="ExternalInput")
    wvT_d = nc.dram_tensor("wvT", [128, CC * C], F16, kind="ExternalInput")
    # one packed tensor for the remaining small inputs:
    # [wkT (512) | y2T (256) | vecs f32 bitcast as 8 fp16 | rows bv+ones (576)]
    smalls_d = nc.dram_tensor("smalls", [128, 1352], F16, kind="ExternalInput")
    out1_d = nc.dram_tensor("out1", [128, CC * N], F16, kind="ExternalOutput")
    out2_d = nc.dram_tensor("out2", [128, CC * N], F16, kind="ExternalOutput")

    with tile.TileContext(nc) as tc:
        with (
            tc.tile_pool(name="const", bufs=1) as const,
            tc.tile_pool(name="qpool", bufs=2) as qpool,
            tc.tile_pool(name="spool", bufs=3) as spool,
            tc.tile_pool(name="apool", bufs=4) as apool,
            tc.tile_pool(name="x1pool", bufs=2) as x1pool,
            tc.tile_pool(name="o1pool", bufs=2) as o1pool,
            tc.tile_pool(name="o2pool", bufs=2) as o2pool,
            tc.tile_pool(name="t2pool", bufs=3) as t2pool,
            tc.tile_pool(name="psq", bufs=2, space="PSUM") as psq,
            tc.tile_pool(name="pse", bufs=1, space="PSUM") as pse,
            tc.tile_pool(name="pst", bufs=1, space="PSUM") as pst,
            tc.tile_pool(name="pso", bufs=2, space="PSUM") as pso,
        ):
            # ---- small weights first: everything in flight round-robins
            # ---- on the SDMA engines, so bulk loads must not lead ----
            wqT_sb = const.tile([128, CC * C4], F16)
            nc.sync.dma_start(out=wqT_sb[:], in_=wqT_d[:])
            x2_sb = []
            x2q0 = const.tile([128, CC * NQ], F16, tag="x2_0", name="x2q0")
            nc.sync.dma_start(out=x2q0[:], in_=x2_d[:, 0 : CC * NQ])
            x2_sb.append(x2q0)
            # one packed load for all remaining small tensors (fewer DMAs
            # in flight = less packet round-robin against x2[0]):
            # [wkT (512) | y2T (256) | vecs as fp16-bitcast (8) | rows (576)]
            smalls_sb = const.tile([128, 1352], F16)
            nc.sync.dma_start(out=smalls_sb[:], in_=smalls_d[:])
            wkT_sb = smalls_sb[:, 0 : CC * C4]
            y2T_sb = smalls_sb[:, CC * C4 : CC * C4 + CC * K]
            vecs_sb = smalls_sb[:, 768:776].bitcast(F32)
            bq_sb = vecs_sb[:, 0:1]
            bk_sb = vecs_sb[:, 1:2]
            sc1_sb = vecs_sb[0:K, 2:3]
            sc2_sb = vecs_sb[0:K, 3:4]
            bv_sb = smalls_sb[0:1, 776 : 776 + C]
            ones_sb = smalls_sb[0:1, 776 + C : 776 + C + K]
            ident = const.tile([128, 128], F16)
            make_identity(nc, ident[:])

            # x1 quarter 0 (gpsimd SWDGE ring): gated behind a 1-element
            # copy that depends on x2[0], so its packets don't round-robin-
            # steal HBM bandwidth from the startup-critical x2[0] load
            # (x1 is only needed once quarter 0's epilogue begins)
            gate = const.tile([1, 1], F16, name="gate")
            nc.gpsimd.tensor_copy(gate[:], x2q0[0:1, 0:1])
            x1_sb = {}
            x1_sb[0] = x1pool.tile([128, CC * NQ], F16, tag="x1t", name="x1t")
            nc.gpsimd.dma_start(out=x1_sb[0][:], in_=x1_d[:, 0 : CC * NQ])

            # ---- kk^T, then a PE warm-up burst while x2[0] is in flight ----
            pkk = pse.tile([C4, K], F32, tag="pse")
            for cc in range(CC):
                nc.tensor.matmul(
                    pkk[:],
                    lhsT=wkT_sb[:, cc * C4 : (cc + 1) * C4],
                    rhs=y2T_sb[:, cc * K : (cc + 1) * K],
                    start=(cc == 0),
                    stop=(cc == CC - 1),
                )
            kkT_sb = const.tile([C4, K], F16)
            nc.scalar.activation(kkT_sb[:], pkk[:], AF.Identity, bias=bk_sb)

            # value-path weights + later x2 quarters issue from the ACT ring
            # AFTER compute ops, so the sequencer naturally holds them back
            # and their packets don't round-robin-steal DMA bandwidth from
            # the startup-critical x2[0]/wqT loads
            wvT_sb = const.tile([128, CC * C], F16)
            nc.scalar.dma_start(out=wvT_sb[:], in_=wvT_d[:])
            y1T_sb = const.tile([128, CC * K], F16)
            nc.scalar.dma_start(out=y1T_sb[:], in_=y1T_d[:])

            pv_sb = []

            def emit_pv():
                # pv1^T, pv2^T: [K, C] = y^T.T @ wvT (+ ones^T bv), scaled.
                # Emitted after quarter 0's attention so the PE doesn't block
                # on the wvT load before starting the q-projection.
                for yT_sb, sc in ((y1T_sb, sc1_sb), (y2T_sb, sc2_sb)):
                    ppv = pso.tile([K, C], F32, tag="pso", name="ppv")
                    for cc in range(CC):
                        nc.tensor.matmul(
                            ppv[:],
                            lhsT=yT_sb[:, cc * K : (cc + 1) * K],
                            rhs=wvT_sb[:, cc * C : (cc + 1) * C],
                            start=(cc == 0),
                            stop=False,
                        )
                    nc.tensor.matmul(
                        ppv[:], lhsT=ones_sb, rhs=bv_sb, start=False, stop=True
                    )
                    pv = const.tile([K, C], F16, tag=f"pv_{len(pv_sb)}", name="pv")
                    nc.scalar.activation(pv[:], ppv[:], AF.Copy, scale=sc)
                    pv_sb.append(pv)

            # ---- pipeline over quarters ----
            for q in range(N // NQ):
                x2q = x2_sb[q]
                # prefetch next x1 quarter on the gpsimd queue
                if q + 1 < N // NQ:
                    x1_sb[q + 1] = x1pool.tile(
                        [128, CC * NQ], F16, tag="x1t", name="x1t"
                    )
                    nc.gpsimd.dma_start(
                        out=x1_sb[q + 1][:],
                        in_=x1_d[:, (q + 1) * CC * NQ : (q + 2) * CC * NQ],
                    )

                # -- attention: q-projection for both halves, cc-outer --
                psum_q = [
                    psq.tile([C4, NT], F32, tag="psq", name="psum_q")
                    for _ in range(2)
                ]
                for cc in range(CC):
                    for h in range(2):
                        nc.tensor.matmul(
                            psum_q[h][:],
                            lhsT=wqT_sb[:, cc * C4 : (cc + 1) * C4],
                            rhs=x2q[:, cc * NQ + h * NT : cc * NQ + h * NT + NT],
                            start=(cc == 0),
                            stop=(cc == CC - 1),
                        )
                if q + 1 < N // NQ:
                    t = const.tile(
                        [128, CC * NQ], F16, tag=f"x2_{q + 1}", name="x2t"
                    )
                    nc.scalar.dma_start(
                        out=t[:], in_=x2_d[:, (q + 1) * CC * NQ : (q + 2) * CC * NQ]
                    )
                    x2_sb.append(t)
                aTs = []
                for h in range(2):
                    qT = qpool.tile([C4, NT], F16, tag="qT")
                    nc.scalar.activation(qT[:], psum_q[h][:], AF.Identity, bias=bq_sb)
                    # energy [n, k] in 128-row chunks: qT_slice.T @ kkT
                    psum_e = pse.tile([128, 4 * K], F32, tag="pse")
                    for s in range(4):
                        nc.tensor.matmul(
                            psum_e[:, s * K : (s + 1) * K],
                            lhsT=qT[:, s * 128 : (s + 1) * 128],
                            rhs=kkT_sb[:],
                            start=True,
                            stop=True,
                        )
                    # softmax(|e|) along k (free dim), no max-subtraction:
                    # |e| <= ~20 here so exp is safely in fp32 range.
                    eexp = spool.tile([128, 4 * K], F32, tag="eexp")
                    nc.vector.tensor_scalar(
                        eexp[:].bitcast(mybir.dt.uint32),
                        psum_e[:].bitcast(mybir.dt.uint32),
                        0x7FFFFFFF,
                        None,
                        op0=OP.bitwise_and,
                    )
                    nc.scalar.activation(eexp[:], eexp[:], AF.Exp)
                    rsum = spool.tile([128, 4], F32, tag="rsum")
                    nc.vector.tensor_reduce(
                        rsum[:],
                        eexp[:].rearrange("p (g d) -> p g d", g=4),
                        axis=AX.X,
                        op=OP.add,
                    )
                    rrec = spool.tile([128, 4], F32, tag="rrec")
                    nc.vector.reciprocal(rrec[:], rsum[:])
                    att = spool.tile([128, 4 * K], F16, tag="att")
                    nc.vector.tensor_tensor(
                        att[:].rearrange("p (g d) -> p g d", g=4),
                        eexp[:].rearrange("p (g d) -> p g d", g=4),
                        rrec[:]
                        .rearrange("p (g o) -> p g o", g=4)
                        .broadcast_to([128, 4, K]),
                        op=OP.mult,
                    )
                    # transpose att [n,k] -> attT [k,n] (fp16 PE transpose)
                    psum_t = pst.tile([K, NT], F16, tag="pst")
                    for s in range(4):
                        nc.tensor.transpose(
                            psum_t[:, s * 128 : (s + 1) * 128],
                            att[:, s * K : (s + 1) * K],
                            ident[:],
                        )
                    aT = apool.tile([K, NT], F16, tag="attT")
                    nc.scalar.copy(aT[:], psum_t[:])
                    aTs.append(aT)

                if q == 0:
                    emit_pv()
                pv1T_sb, pv2T_sb = pv_sb

                # -- output GEMMs + epilogue --
                # PSUM pairs [128, 2*NT] so each epilogue op covers a full NQ.
                # out1: DVE add (psum + x1 -> fp16).  out2: one pair per
                # quarter on DVE, the rest ACT copy + GPSIMD add (keeps DVE,
                # ACT and GpSimd all below the PE wall).
                o1 = o1pool.tile([128, CC * NQ], F16, tag="o1")
                o2 = o2pool.tile([128, CC * NQ], F16, tag="o2")
                for cc in range(CC):
                    o = cc * NQ
                    pos = []
                    for pvT in (pv1T_sb, pv2T_sb):
                        po = pso.tile([128, 2 * NT], F32, tag="pso", name="po")
                        for h in range(2):
                            nc.tensor.matmul(
                                po[:, h * NT : (h + 1) * NT],
                                lhsT=pvT[:, cc * 128 : (cc + 1) * 128],
                                rhs=aTs[h][:],
                                start=True,
                                stop=True,
                            )
                        pos.append(po)
                    nc.vector.tensor_add(
                        o1[:, o : o + NQ], pos[0][:], x1_sb[q][:, o : o + NQ]
                    )
                    if cc == 3:
                        nc.vector.tensor_add(
                            o2[:, o : o + NQ], pos[1][:], x2_sb[q][:, o : o + NQ]
                        )
                    else:
                        t2 = t2pool.tile([128, 2 * NT], F16, tag="t2")
                        nc.scalar.copy(t2[:], pos[1][:])
                        nc.gpsimd.tensor_add(
                            o2[:, o : o + NQ], t2[:], x2_sb[q][:, o : o + NQ]
                        )
                    if cc == 1 or cc == 3:
                        g = cc // 2  # store half-quarters as they complete
                        base = q * CC * NQ + 2 * g * NQ
                        nc.sync.dma_start(
                            out=out1_d[:, base : base + 2 * NQ],
                            in_=o1[:, 2 * g * NQ : (2 * g + 2) * NQ],
                        )
                        nc.scalar.dma_start(
                            out=out2_d[:, base : base + 2 * NQ],
                            in_=o2[:, 2 * g * NQ : (2 * g + 2) * NQ],
                        )
    nc.compile()
    return nc


def _get_nc():
    if "nc" not in _CACHE:
        _CACHE["nc"] = _build_nc()
    return _CACHE["nc"]


def _chunked(a):
    """[C, inner] -> [128, CC*inner] host pack (row chunk cc at cols cc*inner)."""
    c, inner = a.shape
    return np.ascontiguousarray(
        a.reshape(CC, 128, inner).transpose(1, 0, 2).reshape(128, CC * inner)
    )


def kernel(x1, y1, x2, y2, wq, bq, wk, bk, wv, bv, scale, scale1, **run_kwargs):
    x1 = np.asarray(x1, np.float32)
    x2 = np.asarray(x2, np.float32)
    vecs = np.stack(
        [
            np.asarray(bq, np.float32).reshape(C4),
            np.asarray(bk, np.float32).reshape(C4),
            np.full(C4, np.asarray(scale).reshape(-1)[0], np.float32),
            np.full(C4, np.asarray(scale1).reshape(-1)[0], np.float32),
        ],
        axis=1,
    )
    rows = np.concatenate(
        [np.asarray(bv, np.float16).reshape(C), np.ones(K, np.float16)]
    ).reshape(1, C + K)
    smalls = np.zeros((128, 1352), np.float16)
    smalls[:, 0:512] = _chunked(np.asarray(wk, np.float32).T.astype(np.float16))
    smalls[:, 768:776] = np.ascontiguousarray(vecs).view(np.float16)
    smalls[0:1, 776:1352] = np.ascontiguousarray(rows).astype(np.float16)
    shared = {
        "wqT": _chunked(np.asarray(wq, np.float32).T.astype(np.float16)),
        "wvT": _chunked(np.asarray(wv, np.float32).T.astype(np.float16)),
    }
    def _qpack(a):
        # [C, N] -> [128, (q, cc, n)]: each quarter contiguous per partition
        return np.ascontiguousarray(
            a.reshape(CC, 128, N // NQ, NQ).transpose(1, 2, 0, 3).reshape(128, CC * N)
        )

    in_maps = []
    for b in range(B):
        in_maps.append(
            {
                "x1h": _qpack(x1[b].reshape(C, N).astype(np.float16)),
                "x2h": _qpack(x2[b].reshape(C, N).astype(np.float16)),
                "y1T": _chunked(np.asarray(y1[b], np.float32).T.astype(np.float16)),
                "smalls": np.concatenate(
                    [
                        smalls[:, 0:512],
                        _chunked(np.asarray(y2[b], np.float32).T.astype(np.float16)),
                        smalls[:, 768:1352],
                    ],
                    axis=1,
                ),
                **shared,
            }
        )
    nc = _get_nc()
    res = run_bass_kernel_spmd(nc, in_maps, list(range(B)), **run_kwargs)
    _CACHE["last_results"] = res

    def _qunpack(a):
        # [128, (q, cc, n)] -> [C, W, H]
        return (
            a.reshape(128, N // NQ, CC, NQ)
            .transpose(2, 0, 1, 3)
            .reshape(C, N)
            .astype(np.float32)
            .reshape(C, W, H)
        )

    out1 = np.stack([_qunpack(res.results[b]["out1"]) for b in range(B)])
    out2 = np.stack([_qunpack(res.results[b]["out2"]) for b in range(B)])
    return (out1, out2)
